# revision 11
# baseline (speedup 1.0000x reference)
"""Trainium2 Bass kernel for the ODEFunc problem (time-conditioned MLP + exact
divergence of the Jacobian), data-parallel over 8 NeuronCores.

Math (per sample row z):
    x1 = z @ W1[:64] + (b1 + t*W1[64])          # t-column folded into bias
    h1 = silu(x1);  s1 = silu'(x1)
    x2 = h1 @ W2 + b2
    h2 = silu(x2);  s2 = silu'(x2)
    dz = h2 @ W3 + b3
    div = rowsum((s1 @ C) * s2),  C = W2 * (W1[:64].T @ W3.T)
    dlogp_dt = -div
silu'(x) is computed on-device from h = silu(x) and T = tanh(x/2) via
    silu'(x) = (1 + T + h*(1-T)) / 2
so the ACT engine only ever needs the {silu, tanh} LUT set (one table load).
The derivative assembly is a single fused custom-DVE op.

Matmuls run in float32r (single-pass PE mode, ~1e-4 matmul rel err vs the
4-cycles-per-row LOW_HIGH exact-fp32 path).  All weight-derived constants are
packed into one [128, BLOB] DRAM blob so the prologue is a single large DMA.
"""
import sys

if '/opt/trn_rl_repo' not in sys.path:
    sys.path.insert(0, '/opt/trn_rl_repo')

import numpy as np

B, D, H = 16384, 64, 256
N_CORES = 8
BC = B // N_CORES          # 2048 rows per core
G = 1024                   # rows per group (a-tile free dim)
NG = BC // G               # groups per core
NSUB = G // 128            # 128-row subtiles per group

# blob layout (units: f32 elements per partition, [128, BLOB_W])
_OFF_W2 = 0                # [128, 2, 256] -> 512
_OFF_C = 512               # [128, 2, 256] -> 512
_OFF_W3 = 1024             # [128, 2, 64]  -> 128
_OFF_ID = 1152             # [128, 128]    -> 128
_OFF_B3 = 1280             # [128, 512]    -> 512 (b3 tiled 8x)
_OFF_BIAS = 1792           # [128, 8]      -> 8
_OFF_NEG1 = 1800           # [128, 1]      -> 1
BLOB_W = 1801

_compiled = {}


def _register_custom_dve_op():
    """Register SILU_BWD_FUSED_ANT: out = (1 + T + h*(1-T)) * imm2 with
    in0=h, in1=T."""
    import concourse.dve_ops as dve_ops
    from concourse.dve_spec import Spec, Src0, Src1, C2, One

    if any(op.name == "SILU_BWD_FUSED_ANT" for op in dve_ops.OPS):
        return next(op for op in dve_ops.OPS if op.name == "SILU_BWD_FUSED_ANT")

    spec = Spec(
        body=((One + Src1) + Src0 * (One - Src1)) * C2,
        reference=lambda in0, in1, s0, s1, imm2: (
            (1.0 + in1.astype(np.float32)) + in0 * (1.0 - in1)
        ) * imm2,
    )
    op = dve_ops.DveOp(
        "SILU_BWD_FUSED_ANT",
        spec,
        subdim=False,
        uops_sha={"v3": "1dc4e106a000efc1", "v4": "9590f733b321b289"},
    )
    dve_ops.OPS.append(op)
    dve_ops.CUSTOM_DVE_SPECS[op.name] = op.spec
    dve_ops._SUB_OPCODE_FOR_NAME[op.name] = (
        dve_ops._CUSTOM_DVE_ROW_BASE + len(dve_ops.OPS) - 1
    )
    return op


def _build():
    from contextlib import ExitStack

    import concourse.bacc as bacc
    import concourse.tile as tile
    import concourse.mybir as mybir

    silu_bwd = _register_custom_dve_op()

    dt = mybir.dt.float32
    dtr = mybir.dt.float32r
    A = mybir.ActivationFunctionType

    nc = bacc.Bacc("TRN2", target_bir_lowering=False, debug=False,
                   num_devices=N_CORES)

    z_d = nc.dram_tensor("z", [BC, D], dtr, kind="ExternalInput").ap()
    w1_d = nc.dram_tensor("w1", [D, H], dtr, kind="ExternalInput").ap()
    blob_d = nc.dram_tensor("blob", [128, BLOB_W], dtr, kind="ExternalInput").ap()

    dz_d = nc.dram_tensor("dz", [BC, D], dt, kind="ExternalOutput").ap()
    dlp_d = nc.dram_tensor("dlp", [BC], dt, kind="ExternalOutput").ap()

    with tile.TileContext(nc) as tc, ExitStack() as ctx:
        consts = ctx.enter_context(tc.tile_pool(name="consts", bufs=1))
        zin_p = ctx.enter_context(tc.tile_pool(name="zin", bufs=1))
        ztsb_p = ctx.enter_context(tc.tile_pool(name="ztsb", bufs=2))
        act_p = ctx.enter_context(tc.tile_pool(name="acts", bufs=2))
        out_p = ctx.enter_context(tc.tile_pool(name="outs", bufs=2))
        dlp_p = ctx.enter_context(tc.tile_pool(name="dlps", bufs=2))
        # PSUM budget (8 banks): "a" tag 2x[128,1024] = 4 banks,
        # "vt" tag 1x[128,1024] = 2 banks, "misc" tag 2x[<=1 bank] = 2 banks
        ps_a = ctx.enter_context(tc.tile_pool(name="ps_a", bufs=2, space="PSUM"))
        ps_v = ctx.enter_context(tc.tile_pool(name="ps_v", bufs=1, space="PSUM"))
        ps_m = ctx.enter_context(tc.tile_pool(name="ps_m", bufs=2, space="PSUM"))

        # --- constants: one blob DMA + w1 + the full z block ---
        blob = consts.tile([128, BLOB_W], dtr)
        nc.sync.dma_start(blob, blob_d)
        zin = zin_p.tile([128, NG * NSUB, D], dtr)
        nc.scalar.dma_start(zin, z_d.rearrange("(s p) d -> p s d", p=128))
        w1sb = consts.tile([D, H], dtr)
        nc.sync.dma_start(w1sb, w1_d)

        w2sb = blob[:, _OFF_W2:_OFF_W2 + 512].rearrange("p (k n) -> p k n", k=2)
        csb = blob[:, _OFF_C:_OFF_C + 512].rearrange("p (k n) -> p k n", k=2)
        w3sb = blob[:, _OFF_W3:_OFF_W3 + 128].rearrange("p (k n) -> p k n", k=2)
        ident = blob[:, _OFF_ID:_OFF_ID + 128]
        b3sb = blob[:, _OFF_B3:_OFF_B3 + 512].bitcast(dt)
        biassb = blob[:, _OFF_BIAS:_OFF_BIAS + 8].bitcast(dt)
        neg1 = blob[:, _OFF_NEG1:_OFF_NEG1 + 1]

        dz3 = dz_d.rearrange("(g s p) d -> g s p d", g=NG, p=128)

        for g in range(NG):
            # --- z transpose (feature-major z.T in SBUF) ---
            ztsb = ztsb_p.tile([D, G], dtr, name=f"ztsb_{g}")
            for q in range(G // 512):
                ztps = ps_m.tile([D, 512], dt, tag="misc", name=f"ztps_{g}_{q}")
                for s in range(4):
                    nc.tensor.transpose(
                        ztps[:, s * 128:(s + 1) * 128].bitcast(dtr),
                        zin[:, g * NSUB + q * 4 + s, :], ident)
                nc.vector.tensor_copy(ztsb[:, q * 512:(q + 1) * 512],
                                      ztps.bitcast(dtr))

            # --- layer 1 ---
            h1, t1, s1 = {}, {}, {}
            for m in range(2):
                a1 = ps_a.tile([128, G], dt, tag="a", name=f"a1_{m}_{g}")
                for q in range(G // 512):
                    nc.tensor.matmul(
                        a1[:, q * 512:(q + 1) * 512],
                        lhsT=w1sb[:, m * 128:(m + 1) * 128],
                        rhs=ztsb[:, q * 512:(q + 1) * 512],
                        start=True, stop=True)
                h1[m] = act_p.tile([128, G], dtr, tag=f"h1_{m}", name=f"h1_{m}_{g}")
                nc.scalar.activation(h1[m], a1, A.Silu,
                                     bias=biassb[:, 0 + m:1 + m])
                t1[m] = act_p.tile([128, G], dt, tag=f"t1_{m}", name=f"t1_{m}_{g}")
                nc.scalar.activation(t1[m], a1, A.Tanh,
                                     bias=biassb[:, 2 + m:3 + m], scale=0.5)
                s1[m] = act_p.tile([128, G], dtr, tag=f"s1_{m}", name=f"s1_{m}_{g}")
                nc.vector._custom_dve(silu_bwd, out=s1[m][:], in0=h1[m][:],
                                      in1=t1[m][:], imm2=0.5)

            # --- layer 2 ---
            h2, t2, s2 = {}, {}, {}
            for m in range(2):
                a2 = ps_a.tile([128, G], dt, tag="a", name=f"a2_{m}_{g}")
                for q in range(G // 512):
                    for k in range(2):
                        nc.tensor.matmul(
                            a2[:, q * 512:(q + 1) * 512],
                            lhsT=w2sb[:, k, m * 128:(m + 1) * 128],
                            rhs=h1[k][:, q * 512:(q + 1) * 512],
                            start=(k == 0), stop=(k == 1))
                h2[m] = act_p.tile([128, G], dtr, tag=f"h2_{m}", name=f"h2_{m}_{g}")
                nc.scalar.activation(h2[m], a2, A.Silu,
                                     bias=biassb[:, 4 + m:5 + m])
                t2[m] = act_p.tile([128, G], dt, tag=f"t2_{m}", name=f"t2_{m}_{g}")
                nc.scalar.activation(t2[m], a2, A.Tanh,
                                     bias=biassb[:, 6 + m:7 + m], scale=0.5)
                s2[m] = act_p.tile([128, G], dtr, tag=f"s2_{m}", name=f"s2_{m}_{g}")
                nc.vector._custom_dve(silu_bwd, out=s2[m][:], in0=h2[m][:],
                                      in1=t2[m][:], imm2=0.5)

            # --- layer 3: feature-major outT, then PE-transpose to batch-major
            outT = ps_v.tile([64, G], dt, tag="vt", name=f"outT_{g}")
            for q in range(G // 512):
                for k in range(2):
                    nc.tensor.matmul(
                        outT[:, q * 512:(q + 1) * 512],
                        lhsT=w3sb[:, k, :],
                        rhs=h2[k][:, q * 512:(q + 1) * 512],
                        start=(k == 0), stop=(k == 1))
            outTsb = out_p.tile([64, G], dtr, tag="outTsb", name=f"outTsb_{g}")
            nc.vector.tensor_copy(outTsb, outT.bitcast(dtr))
            for q in range(G // 512):
                outB = ps_m.tile([128, 256], dt, tag="misc", name=f"outB_{g}_{q}")
                for s in range(4):
                    nc.tensor.transpose(
                        outB[:, s * 64:(s + 1) * 64].bitcast(dtr),
                        outTsb[:, (q * 4 + s) * 128:(q * 4 + s + 1) * 128],
                        ident[:64, :64])
                outsb = out_p.tile([128, 256], dt, tag="outsb",
                                   name=f"outsb_{g}_{q}")
                nc.vector.tensor_add(outsb, outB, b3sb[:, :256])
                nc.sync.dma_start(
                    dz3[g, q * 4:(q + 1) * 4].rearrange("s p d -> p s d"),
                    outsb[:].rearrange("p (s d) -> p s d", d=D))

            # --- divergence: v = C^T-chunks @ s1, w = v*s2, dlp = -colsum ---
            w = {}
            for m in range(2):
                vps = ps_v.tile([128, G], dt, tag="vt", name=f"v_{m}_{g}")
                for q in range(G // 512):
                    for k in range(2):
                        nc.tensor.matmul(
                            vps[:, q * 512:(q + 1) * 512],
                            lhsT=csb[:, k, m * 128:(m + 1) * 128],
                            rhs=s1[k][:, q * 512:(q + 1) * 512],
                            start=(k == 0), stop=(k == 1))
                w[m] = act_p.tile([128, G], dtr, tag=f"w_{m}", name=f"w_{m}_{g}")
                nc.vector.tensor_mul(w[m], vps, s2[m])

            for q in range(G // 512):
                divps = ps_m.tile([1, 512], dt, tag="misc", name=f"div_{g}_{q}")
                for k in range(2):
                    nc.tensor.matmul(
                        divps,
                        lhsT=neg1,
                        rhs=w[k][:, q * 512:(q + 1) * 512],
                        start=(k == 0), stop=(k == 1))
                dlpsb = dlp_p.tile([1, 512], dt, name=f"dlpsb_{g}_{q}")
                nc.scalar.copy(dlpsb, divps)
                nc.sync.dma_start(
                    dlp_d[g * G + q * 512:g * G + (q + 1) * 512]
                    .rearrange("(a b) -> a b", a=1),
                    dlpsb)

    nc.compile()
    return nc


def _get_compiled():
    if "nc" not in _compiled:
        _compiled["nc"] = _build()
    return _compiled["nc"]


def make_in_maps(t, z, logp, W1, b1, W2, b2, W3, b3):
    t = np.asarray(t, np.float32)
    z = np.ascontiguousarray(np.asarray(z, np.float32))
    W1 = np.asarray(W1, np.float32)
    b1 = np.asarray(b1, np.float32)
    W2 = np.asarray(W2, np.float32)
    b2 = np.asarray(b2, np.float32)
    W3 = np.asarray(W3, np.float32)
    b3 = np.asarray(b3, np.float32)

    b1e = b1 + t[0] * W1[D]
    cmat = W2 * (W1[:D].T @ W3.T)

    blob = np.zeros((128, BLOB_W), np.float32)
    blob[:, _OFF_W2:_OFF_W2 + 512] = np.concatenate([W2[:128], W2[128:]], axis=1)
    blob[:, _OFF_C:_OFF_C + 512] = np.concatenate([cmat[:128], cmat[128:]], axis=1)
    blob[:, _OFF_W3:_OFF_W3 + 128] = np.concatenate([W3[:128], W3[128:]], axis=1)
    blob[:, _OFF_ID:_OFF_ID + 128] = np.eye(128, dtype=np.float32)
    blob[:, _OFF_B3:_OFF_B3 + 512] = np.tile(b3, (128, 8))
    blob[:, _OFF_BIAS:_OFF_BIAS + 8] = np.stack(
        [b1e[:128], b1e[128:], .5 * b1e[:128], .5 * b1e[128:],
         b2[:128], b2[128:], .5 * b2[:128], .5 * b2[128:]], axis=1)
    blob[:, _OFF_NEG1] = -1.0
    blob = np.ascontiguousarray(blob)
    w1c = np.ascontiguousarray(W1[:D])

    shared = {"w1": w1c, "blob": blob}
    return [dict(shared, z=np.ascontiguousarray(z[c * BC:(c + 1) * BC]))
            for c in range(N_CORES)]


def kernel(t, z, logp, W1, b1, W2, b2, W3, b3):
    from concourse.bass_utils import run_bass_kernel_spmd
    in_maps = make_in_maps(t, z, logp, W1, b1, W2, b2, W3, b3)
    nc = _get_compiled()
    res = run_bass_kernel_spmd(nc, in_maps, core_ids=list(range(N_CORES)))

    dz = np.concatenate([res.results[c]["dz"] for c in range(N_CORES)], axis=0)
    dlp = np.concatenate([res.results[c]["dlp"] for c in range(N_CORES)], axis=0)
    return dz, dlp
